# revision 6
# baseline (speedup 1.0000x reference)
"""GraphSage (2-layer, mean aggr) on 8 trn2 NeuronCores — V3.

Scheme (dst-sharded edge-parallel, 4-queue SWDGE gather, piecewise AllGather):
  - Nodes padded to 50176 = 8 * 6272; core c owns dst nodes [c*6272, (c+1)*6272).
  - dst tiles of TD=128 (49/core); edges bucketed by (tile, src-group) with 4
    src groups = 4 SWDGE queues (disjoint Q7 worker pairs run concurrently).
    Layer 1 groups: global src quarters (x input, base offsets q*12544).
    Layer 2 groups: local-slice pieces r (src -> (core k, local l), r by l in
    [0,1536,3072,4608,6272)); piece r of h is AllGathered into a contiguous
    DRAM region as soon as the 12-13 dst tiles producing it finish, so layer-2
    gathers overlap layer-1 tail + collective.
  - Each (tile, group) list padded to a multiple of 128 (shared chunk
    structure = max count across cores; pad: idx=0, dst_local=-1).
  - dma_gather of 256B f32 rows, one call per (tile-batch, group) on queue g.
  - msgs downcast f32->bf16 (alternating ScalarE/DVE); one-hot bf16 on DVE
    (is_equal vs materialized iota); scatter matmul lhsT=msg [128e, 64f],
    rhs=onehot [128e, 128d] -> psum aggT [64f, 128d] (bf16 = 1 cyc/col).
  - Mean fold: aggT = psum * rec_rep (DVE); per-tile GEMM
    hT = W1l.T@aggT + W1r.T@xT (+b1); ELU = relu(z) - relu(1 - exp(z))
    (3 scalar activations + one DVE subtract).
"""

import sys

sys.path.insert(0, "/opt/trn_rl_repo")

import numpy as np

import concourse.bacc as bacc
import concourse.mybir as mybir
import concourse.tile as tile
from concourse.bass_utils import run_bass_kernel_spmd

N, E, D, H, O = 50000, 800000, 64, 64, 16
NCORES = 8
NSH = 6272                  # dst nodes per core
NPAD = NSH * NCORES         # 50176
TD = 128                    # dst tile size
NT = NSH // TD              # 49 dst tiles per core
CH = 128                    # edges per matmul chunk
NQ = 4                      # src groups == SWDGE queues
QW = NPAD // NQ             # 12544 L1 quarter width
TB = 2                      # dst tiles per gather batch
SUB = 24                    # max chunks per one-hot build op
PIECE_TILES = (12, 12, 12, 13)          # dst tiles per h piece
PIECE_B = (0, 1536, 3072, 4608, 6272)   # local-row piece bounds

f32 = mybir.dt.float32
bf16 = mybir.dt.bfloat16
i16 = mybir.dt.int16

DEBUG_L1_ONLY = False


def _preprocess(edge_index):
    src = np.asarray(edge_index[0], dtype=np.int64)
    dst = np.asarray(edge_index[1], dtype=np.int64)
    core = dst // NSH
    ldst = dst - core * NSH
    tid = ldst // TD
    tloc = (ldst % TD).astype(np.float32)

    def make_layer(grp, gidx):
        key = ((core * NT + tid) * NQ + grp)
        order = np.argsort(key, kind="stable")
        gidx_s = gidx[order]
        tloc_s = tloc[order]
        counts = np.bincount(key[order], minlength=NCORES * NT * NQ).reshape(
            NCORES, NT, NQ
        )
        starts = np.zeros(NCORES * NT * NQ + 1, dtype=np.int64)
        np.cumsum(counts.reshape(-1), out=starts[1:])
        cmax = counts.max(axis=0)
        nch = ((cmax + CH - 1) // CH).astype(np.int64)

        batches = []
        gchunk = 0
        for b0 in range(0, NT, TB):
            tids = list(range(b0, min(b0 + TB, NT)))
            qinfo = []
            tcols = {t: [] for t in tids}
            col = 0
            for g in range(NQ):
                g0 = col
                for t in tids:
                    n = int(nch[t, g])
                    if n:
                        tcols[t].append((col, n))
                    col += n
                qinfo.append((g0, col - g0))
            batches.append(
                dict(tids=tids, qinfo=qinfo, tcols=tcols,
                     nchb=col, gchunk0=gchunk)
            )
            gchunk += col
        NCHT = gchunk
        S = NCHT * CH

        idx_wraps, dl_wraps = [], []
        for k in range(NCORES):
            idx_flat = np.zeros(S, dtype=np.int16)
            dl_flat = np.full(S, -1.0, dtype=np.float32)
            for b in batches:
                for g in range(NQ):
                    coff = b["gchunk0"] + b["qinfo"][g][0]
                    for t in b["tids"]:
                        n = int(nch[t, g])
                        if n == 0:
                            continue
                        ki = (k * NT + t) * NQ + g
                        cnt = int(counts[k, t, g])
                        s0 = coff * CH
                        e0 = int(starts[ki])
                        idx_flat[s0 : s0 + cnt] = gidx_s[e0 : e0 + cnt]
                        dl_flat[s0 : s0 + cnt] = tloc_s[e0 : e0 + cnt]
                        coff += n
            idx_wraps.append(
                np.ascontiguousarray(
                    np.tile(idx_flat.reshape(S // 16, 16).T, (8, 1))
                )
            )
            dl_wraps.append(
                np.ascontiguousarray(dl_flat.reshape(NCHT, CH).T)
            )
        return dict(batches=batches, S=S, NCHT=NCHT,
                    idx=idx_wraps, dl=dl_wraps)

    # layer 1: global src quarters
    grp1 = src // QW
    gidx1 = (src - grp1 * QW).astype(np.int16)
    L1 = make_layer(grp1, gidx1)

    # layer 2: local-slice pieces
    k2 = src // NSH
    l2 = src - k2 * NSH
    grp2 = np.digitize(l2, list(PIECE_B[1:4]))
    pw = np.array([PIECE_B[r + 1] - PIECE_B[r] for r in range(NQ)])
    off = np.array(PIECE_B[:4])
    gidx2 = (k2 * pw[grp2] + (l2 - off[grp2])).astype(np.int16)
    L2 = make_layer(grp2, gidx2)

    recs = []
    for k in range(NCORES):
        deg = np.bincount(ldst[core == k], minlength=NSH).astype(np.float32)
        rec = (1.0 / np.maximum(deg, 1.0)).astype(np.float32)
        recs.append(np.ascontiguousarray(np.tile(rec[None, :], (64, 1))))

    return L1, L2, recs


def _build(nc, L1, L2):
    x_d = nc.dram_tensor("x", [N, D], f32, kind="ExternalInput")
    xT_d = nc.dram_tensor("xT", [D, NSH], bf16, kind="ExternalInput")
    idx1_d = nc.dram_tensor("idx1", [128, L1["S"] // 16], i16, kind="ExternalInput")
    dl1_d = nc.dram_tensor("dl1", [128, L1["NCHT"]], bf16, kind="ExternalInput")
    idx2_d = nc.dram_tensor("idx2", [128, L2["S"] // 16], i16, kind="ExternalInput")
    dl2_d = nc.dram_tensor("dl2", [128, L2["NCHT"]], bf16, kind="ExternalInput")
    rec_d = nc.dram_tensor("rec", [64, NSH], f32, kind="ExternalInput")
    w1l_d = nc.dram_tensor("w1l", [D, H], bf16, kind="ExternalInput")
    w1r_d = nc.dram_tensor("w1r", [D, H], bf16, kind="ExternalInput")
    b1_d = nc.dram_tensor("b1", [H, 1], f32, kind="ExternalInput")
    w2l_d = nc.dram_tensor("w2l", [H, O], bf16, kind="ExternalInput")
    w2r_d = nc.dram_tensor("w2r", [H, O], bf16, kind="ExternalInput")
    b2_d = nc.dram_tensor("b2", [O, 1], f32, kind="ExternalInput")
    iota_d = nc.dram_tensor("iotar", [128, SUB * TD], bf16, kind="ExternalInput")
    ident_d = nc.dram_tensor("ident", [D, D], bf16, kind="ExternalInput")
    outT_d = nc.dram_tensor("outT", [O, NSH], f32, kind="ExternalOutput")
    hout_d = (
        nc.dram_tensor("hout", [NSH, D], f32, kind="ExternalOutput")
        if DEBUG_L1_ONLY
        else None
    )

    with tile.TileContext(nc, num_cores=NCORES) as tc:
        with (
            tc.tile_pool(name="const", bufs=1) as cpool,
            tc.tile_pool(name="msgq", bufs=3) as mqpool,
            tc.tile_pool(name="msgb", bufs=3) as mbpool,
            tc.tile_pool(name="oh", bufs=3) as ohpool,
            tc.tile_pool(name="aggT", bufs=4) as aggpool,
            tc.tile_pool(name="hT", bufs=NT) as hTpool,
            tc.tile_pool(name="small", bufs=4) as spool,
            tc.tile_pool(name="ps_sc", bufs=4, space="PSUM") as ps_sc,
            tc.tile_pool(name="ps_mm", bufs=2, space="PSUM") as ps_mm,
            tc.tile_pool(name="ps_tr", bufs=2, space="PSUM") as ps_tr,
            tc.tile_pool(name="dram", bufs=1, space="DRAM") as dpool,
        ):
            idx1_sb = cpool.tile([128, L1["S"] // 16], i16, tag="idx1")
            nc.sync.dma_start(idx1_sb[:], idx1_d[:])
            dl1_sb = cpool.tile([128, L1["NCHT"]], bf16, tag="dl1")
            nc.sync.dma_start(dl1_sb[:], dl1_d[:])
            idx2_sb = cpool.tile([128, L2["S"] // 16], i16, tag="idx2")
            nc.sync.dma_start(idx2_sb[:], idx2_d[:])
            dl2_sb = cpool.tile([128, L2["NCHT"]], bf16, tag="dl2")
            nc.sync.dma_start(dl2_sb[:], dl2_d[:])
            rec_sb = cpool.tile([64, NSH], f32, tag="rec")
            nc.sync.dma_start(rec_sb[:], rec_d[:])
            xT_sb = cpool.tile([D, NSH], bf16, tag="xT")
            nc.sync.dma_start(xT_sb[:], xT_d[:])
            w1l_sb = cpool.tile([D, H], bf16, tag="w1l")
            nc.sync.dma_start(w1l_sb[:], w1l_d[:])
            w1r_sb = cpool.tile([D, H], bf16, tag="w1r")
            nc.sync.dma_start(w1r_sb[:], w1r_d[:])
            b1_sb = cpool.tile([H, 1], f32, tag="b1")
            nc.sync.dma_start(b1_sb[:], b1_d[:])
            w2l_sb = cpool.tile([H, O], bf16, tag="w2l")
            nc.sync.dma_start(w2l_sb[:], w2l_d[:])
            w2r_sb = cpool.tile([H, O], bf16, tag="w2r")
            nc.sync.dma_start(w2r_sb[:], w2r_d[:])
            b2_sb = cpool.tile([O, 1], f32, tag="b2")
            nc.sync.dma_start(b2_sb[:], b2_d[:])
            iota_sb = cpool.tile([128, SUB * TD], bf16, tag="iotar")
            nc.sync.dma_start(iota_sb[:], iota_d[:])
            id_sb = cpool.tile([D, D], bf16, tag="ident")
            nc.sync.dma_start(id_sb[:], ident_d[:])

            # per-piece h shards and AllGathered pieces
            h_sh_r = [
                dpool.tile([PIECE_B[r + 1] - PIECE_B[r], D], f32,
                           tag=f"hsh{r}", name=f"hsh{r}")
                for r in range(NQ)
            ]
            h_piece = [
                dpool.tile([NCORES * (PIECE_B[r + 1] - PIECE_B[r]), D], f32,
                           tag=f"hp{r}", name=f"hp{r}")
                for r in range(NQ)
            ]

            def layer(LS, idx_sb, dl_sb, srcs, wl, wr, bias, is_l1):
                out_tiles = []
                batches = LS["batches"]
                for bi, b in enumerate(batches):
                    nchb = b["nchb"]
                    gc0 = b["gchunk0"]
                    mq = []
                    for g in range(NQ):
                        g0, gn = b["qinfo"][g]
                        if gn == 0:
                            mq.append(None)
                            continue
                        m = mqpool.tile([128, gn, D], f32, tag=f"mq{g}")
                        c0 = (gc0 + g0) * 8
                        nc.gpsimd.dma_gather(
                            m[:], srcs[g], idx_sb[:, c0 : c0 + gn * 8],
                            gn * CH, gn * CH, D,
                            single_packet=False, queue_num=g,
                        )
                        mq.append(m)
                    mb = mbpool.tile([128, nchb, D], bf16, tag="mb")
                    for g in range(NQ):
                        g0, gn = b["qinfo"][g]
                        if gn == 0:
                            continue
                        if (bi + g) % 2 == 0:
                            nc.scalar.activation(
                                mb[:, g0 : g0 + gn, :], mq[g][:],
                                mybir.ActivationFunctionType.Copy,
                            )
                        else:
                            nc.vector.tensor_copy(
                                out=mb[:, g0 : g0 + gn, :], in_=mq[g][:]
                            )
                    ohs = []
                    for j0 in range(0, nchb, SUB):
                        kk = min(SUB, nchb - j0)
                        oh = ohpool.tile([128, SUB, TD], bf16, tag="oh")
                        dl_b = (
                            dl_sb[:, gc0 + j0 : gc0 + j0 + kk]
                            .rearrange("p (k o) -> p k o", o=1)
                            .to_broadcast((128, kk, TD))
                        )
                        io_b = iota_sb[:, 0 : kk * TD].rearrange(
                            "p (k t) -> p k t", t=TD
                        )
                        nc.vector.tensor_tensor(
                            out=oh[:, 0:kk, :], in0=dl_b, in1=io_b,
                            op=mybir.AluOpType.is_equal,
                        )
                        ohs.append(oh)

                    for t in b["tids"]:
                        cols = []
                        for c0, n in b["tcols"][t]:
                            cols.extend(range(c0, c0 + n))
                        ps = ps_sc.tile([D, TD], f32, tag="ps")
                        for ji, j in enumerate(cols):
                            nc.tensor.matmul(
                                out=ps[:],
                                lhsT=mb[:, j, :],
                                rhs=ohs[j // SUB][:, j % SUB, :],
                                start=(ji == 0),
                                stop=(ji == len(cols) - 1),
                            )
                        nsl = slice(t * TD, (t + 1) * TD)
                        aggT = aggpool.tile([D, TD], bf16, tag="aggT")
                        nc.vector.tensor_tensor(
                            out=aggT[:], in0=ps[:], in1=rec_sb[:, nsl],
                            op=mybir.AluOpType.mult,
                        )
                        Dout = H if is_l1 else O
                        ph = ps_mm.tile([Dout, TD], f32, tag="mm")
                        nc.tensor.matmul(
                            out=ph[:], lhsT=wl[:], rhs=aggT[:],
                            start=True, stop=False,
                        )
                        rhs2 = xT_sb[:, nsl] if is_l1 else hT_tiles[t][:]
                        nc.tensor.matmul(
                            out=ph[:], lhsT=wr[:], rhs=rhs2,
                            start=False, stop=True,
                        )
                        if is_l1:
                            # ELU(z+b) = relu(z+b) - relu(1 - exp(z+b))
                            et = spool.tile([H, TD], bf16, tag="et")
                            nc.scalar.activation(
                                et[:], ph[:],
                                mybir.ActivationFunctionType.Exp,
                                bias=bias[:, 0:1],
                            )
                            hT = hTpool.tile([H, TD], bf16, tag="hT")
                            nc.scalar.activation(
                                hT[:], ph[:],
                                mybir.ActivationFunctionType.Relu,
                                bias=bias[:, 0:1],
                            )
                            et2 = spool.tile([H, TD], bf16, tag="et2")
                            nc.scalar.activation(
                                et2[:], et[:],
                                mybir.ActivationFunctionType.Relu,
                                bias=1.0, scale=-1.0,
                            )
                            nc.vector.tensor_tensor(
                                out=hT[:], in0=hT[:], in1=et2[:],
                                op=mybir.AluOpType.subtract,
                            )
                            out_tiles.append(hT)
                            pn = ps_tr.tile([TD, H], bf16, tag="pn")
                            nc.tensor.transpose(
                                out=pn[:], in_=hT[:], identity=id_sb[:]
                            )
                            hs = spool.tile([TD, H], f32, tag="hs")
                            nc.vector.tensor_copy(out=hs[:], in_=pn[:])
                            r = min(t // 12, 3)
                            r0 = t * TD - PIECE_B[r]
                            if DEBUG_L1_ONLY:
                                nc.sync.dma_start(hout_d.ap()[nsl, :], hs[:])
                            else:
                                nc.sync.dma_start(
                                    h_sh_r[r][r0 : r0 + TD, :], hs[:]
                                )
                                if t == sum(PIECE_TILES[: r + 1]) - 1:
                                    nc.gpsimd.collective_compute(
                                        "AllGather",
                                        mybir.AluOpType.bypass,
                                        replica_groups=[list(range(NCORES))],
                                        ins=[h_sh_r[r][:]],
                                        outs=[h_piece[r][:]],
                                    )
                        else:
                            ot = spool.tile([O, TD], f32, tag="ot")
                            nc.vector.tensor_scalar(
                                out=ot[:], in0=ph[:],
                                scalar1=bias[:, 0:1], scalar2=None,
                                op0=mybir.AluOpType.add,
                            )
                            nc.sync.dma_start(outT_d.ap()[:, nsl], ot[:])
                return out_tiles

            x_ap = x_d.ap()
            xsrcs = [x_ap[g * QW : min((g + 1) * QW, N), :] for g in range(NQ)]
            hT_tiles = layer(L1, idx1_sb, dl1_sb, xsrcs, w1l_sb, w1r_sb,
                             b1_sb, True)

            if not DEBUG_L1_ONLY:
                hsrcs = [h_piece[r][:] for r in range(NQ)]
                layer(L2, idx2_sb, dl2_sb, hsrcs, w2l_sb, w2r_sb, b2_sb, False)

    return nc


def _run(inputs, trace=False):
    x = np.ascontiguousarray(np.asarray(inputs["x"], dtype=np.float32))
    ei = np.asarray(inputs["edge_index"])
    W1l = np.asarray(inputs["W1l"], np.float32)
    W1r = np.asarray(inputs["W1r"], np.float32)
    b1 = np.asarray(inputs["b1"], np.float32).reshape(H, 1).copy()
    W2l = np.asarray(inputs["W2l"], np.float32)
    W2r = np.asarray(inputs["W2r"], np.float32)
    b2 = np.asarray(inputs["b2"], np.float32).reshape(O, 1).copy()

    L1, L2, recs = _preprocess(ei)

    nc = bacc.Bacc(
        "TRN2",
        target_bir_lowering=False,
        debug=False,
        enable_asserts=False,
        num_devices=NCORES,
        num_swdge_queues=NQ,
    )
    _build(nc, L1, L2)
    nc.compile()

    iota = np.ascontiguousarray(
        np.tile(np.arange(TD, dtype=np.float32), (128, SUB))
    )
    ident = np.eye(D, dtype=np.float32)

    def tobf(a):
        import jax.numpy as jnp

        return np.asarray(jnp.asarray(a, dtype=jnp.bfloat16))

    in_maps = []
    for k in range(NCORES):
        lo = k * NSH
        hi = min((k + 1) * NSH, N)
        xt = np.zeros((D, NSH), dtype=np.float32)
        xt[:, 0 : hi - lo] = x[lo:hi].T
        in_maps.append(
            dict(
                x=x,
                xT=tobf(xt),
                idx1=L1["idx"][k],
                dl1=tobf(L1["dl"][k]),
                idx2=L2["idx"][k],
                dl2=tobf(L2["dl"][k]),
                rec=recs[k],
                w1l=tobf(W1l),
                w1r=tobf(W1r),
                b1=b1,
                w2l=tobf(W2l),
                w2r=tobf(W2r),
                b2=b2,
                iotar=tobf(iota),
                ident=tobf(ident),
            )
        )

    res = run_bass_kernel_spmd(
        nc, in_maps, core_ids=list(range(NCORES)), trace=trace
    )
    if DEBUG_L1_ONLY:
        outs = [res.results[k]["hout"] for k in range(NCORES)]
        full = np.concatenate(outs, axis=0)[:N]
        return np.ascontiguousarray(full.astype(np.float32)), res
    outs = [res.results[k]["outT"] for k in range(NCORES)]
    full = np.concatenate([o.T for o in outs], axis=0)[:N]
    return np.ascontiguousarray(full.astype(np.float32)), res


def kernel(**inputs):
    out, _ = _run(inputs, trace=False)
    return out
